# revision 7
# baseline (speedup 1.0000x reference)
"""K-means argmin kernel for Trainium2 (8 NeuronCores, data-parallel over N).

Problem: x [131072, 512] f32, cluster_centers [2048, 512] f32.
Output: argmin_k ||x_n - c_k||_2  -> int32 [131072].

Math: argmin_k (x2 + c2 - 2 x.c) == argmax_k (x.c - c2/2)   (x2 is per-row const)

Per-core layout (N sharded 8-ways -> 16384 rows/core, 128 tiles of 128 rows):
  - c is transposed once on-device via PE transpose into cT[db] [128d, 2048k], db=0..3
  - bias[p,k] = -0.5*sum_d c[k,d]^2 broadcast to all partitions, computed with a
    (-0.5)-filled stationary matmul over elementwise-squared cT
  - per x-tile: DMA [128,512] -> PE-transpose to xT -> 16 matmuls accumulate
    scores[128,2048] in PSUM -> DVE adds bias -> vector.max + vector.max_index
    give the argmax index; indices accumulate in SBUF, one DMA out at the end.

MODE:
  "fp32"   - true fp32 matmuls (4 PE passes/row, exact-ish)
  "fp32r"  - single-pass fp32 (operands truncated to ~fp22 by the PE)
  "bf16x3" - split x,c into bf16 hi+lo, 3 passes (hi*hi + hi*lo + lo*hi)
"""

import os
import sys

sys.path.insert(0, "/opt/trn_rl_repo")

import numpy as np

from concourse import bacc, mybir, tile
from concourse.bass import ts
from concourse.bass_utils import run_bass_kernel_spmd
from concourse.masks import make_identity

N, K, D = 131072, 2048, 512
N_CORES = 8
N_LOC = N // N_CORES          # 16384 rows per core
P = 128                        # partitions
DB = D // P                    # 4 contraction steps
KC = K // 512                  # 4 psum bank chunks of 512

F32 = mybir.dt.float32
F32R = mybir.dt.float32r
BF16 = mybir.dt.bfloat16
U32 = mybir.dt.uint32

MODE = os.environ.get("KM_MODE", "bf16x3")
FUSE = os.environ.get("KM_FUSE", "0") == "1"


def _round_fp22(a: np.ndarray) -> np.ndarray:
    """Round f32 mantissa to 13 bits (nearest) so the PE's fp32r truncation
    to ~fp22 becomes exact, removing truncation bias."""
    u = a.view(np.uint32) if a.flags["C_CONTIGUOUS"] else \
        np.ascontiguousarray(a).view(np.uint32)
    r = ((u.astype(np.uint64) + 0x200) & ~np.uint64(0x3FF)).astype(np.uint32)
    return r.view(np.float32).reshape(a.shape)


def build_nc(mode: str = MODE, n_tiles: int = N_LOC // P):
    if mode == "fp32rr":          # same device program; host pre-rounds inputs
        mode = "fp32r"
    nc = bacc.Bacc("TRN2", target_bir_lowering=False, debug=False,
                   num_devices=N_CORES)

    x_d = nc.dram_tensor("x", [n_tiles * P, D], F32, kind="ExternalInput")
    c_d = nc.dram_tensor("cc", [K, D], F32, kind="ExternalInput")
    o_d = nc.dram_tensor("out", [P, n_tiles * 8], U32, kind="ExternalOutput")

    with tile.TileContext(nc) as tc:
        with (
            tc.tile_pool(name="const", bufs=1) as cpool,
            tc.tile_pool(name="work", bufs=3) as wpool,
            tc.tile_pool(name="scores", bufs=2) as spool,
            tc.tile_pool(name="psum_sc", bufs=1, space="PSUM") as psc,
            tc.tile_pool(name="psum_tp", bufs=2, space="PSUM") as ptp,
        ):
            ident = cpool.tile([P, P], F32)
            make_identity(nc, ident)
            halfneg = cpool.tile([P, P], F32)
            nc.vector.memset(halfneg, -0.5)

            # ---- transpose c into cT[db] (f32), and bf16 hi/lo if needed ----
            cT = [cpool.tile([P, K], F32, name=f"cT{i}") for i in range(DB)]
            for kt in range(K // P):
                c_nat = wpool.tile([P, D], F32, tag="c_nat")
                nc.sync.dma_start(c_nat[:], c_d.ap()[ts(kt, P), :])
                for db in range(DB):
                    tp = ptp.tile([P, P], F32, tag="tp_c")
                    nc.tensor.transpose(tp[:], c_nat[:, ts(db, P)], ident[:])
                    nc.vector.tensor_copy(cT[db][:, ts(kt, P)], tp[:])

            # ---- bias[p,k] = -0.5 * sum_d cT[d,k]^2 (same for all p) ----
            bias_sb = cpool.tile([P, K], F32)
            sqs = []
            for db in range(DB):
                sq = wpool.tile([P, K], F32, tag=f"sq{db}")
                nc.vector.tensor_mul(sq[:], cT[db][:], cT[db][:])
                sqs.append(sq)
            bias_ps = psc.tile([P, K], F32, tag="score_ps")
            for kc in range(KC):
                for db in range(DB):
                    nc.tensor.matmul(bias_ps[:, ts(kc, 512)], halfneg[:],
                                     sqs[db][:, ts(kc, 512)],
                                     start=(db == 0), stop=(db == DB - 1))
            nc.vector.tensor_copy(bias_sb[:], bias_ps[:])

            if mode == "bf16x3":
                cT_h = [cpool.tile([P, K], BF16, name=f"cTh{i}") for i in range(DB)]
                cT_l = [cpool.tile([P, K], BF16, name=f"cTl{i}") for i in range(DB)]
                for db in range(DB):
                    nc.vector.tensor_copy(cT_h[db][:], cT[db][:])
                    nc.vector.tensor_sub(cT_l[db][:], cT[db][:], cT_h[db][:])

            idx_acc = cpool.tile([P, n_tiles * 8], U32)

            # ---- main loop over x tiles ----
            for t in range(n_tiles):
                x_nat = wpool.tile([P, D], F32, tag="x_nat")
                nc.sync.dma_start(x_nat[:], x_d.ap()[ts(t, P), :])

                tpx = ptp.tile([P, D], F32, tag="tp_x")
                for db in range(DB):
                    nc.tensor.transpose(tpx[:, ts(db, P)], x_nat[:, ts(db, P)],
                                        ident[:])
                if mode == "bf16x3":
                    xh = wpool.tile([P, D], BF16, tag="xh")
                    xl = wpool.tile([P, D], BF16, tag="xl")
                    nc.vector.tensor_copy(xh[:], tpx[:])
                    nc.vector.tensor_sub(xl[:], tpx[:], xh[:])
                else:
                    xT = wpool.tile([P, D], F32, tag="xT")
                    nc.vector.tensor_copy(xT[:], tpx[:])

                score_ps = psc.tile([P, K], F32, tag="score_ps")
                for kc in range(KC):
                    if mode == "bf16x3":
                        passes = []
                        for db in range(DB):
                            passes += [
                                (xh[:, ts(db, P)], cT_h[db][:, ts(kc, 512)]),
                                (xh[:, ts(db, P)], cT_l[db][:, ts(kc, 512)]),
                                (xl[:, ts(db, P)], cT_h[db][:, ts(kc, 512)]),
                            ]
                        for i, (lhsT, rhs) in enumerate(passes):
                            nc.tensor.matmul(score_ps[:, ts(kc, 512)], lhsT, rhs,
                                             start=(i == 0),
                                             stop=(i == len(passes) - 1))
                    else:
                        for db in range(DB):
                            lhsT = xT[:, ts(db, P)]
                            rhs = cT[db][:, ts(kc, 512)]
                            if mode == "fp32r":
                                lhsT = lhsT.bitcast(F32R)
                                rhs = rhs.bitcast(F32R)
                            nc.tensor.matmul(score_ps[:, ts(kc, 512)], lhsT, rhs,
                                             start=(db == 0), stop=(db == DB - 1))

                scores = spool.tile([P, K], F32, tag="scores")
                if FUSE:
                    # one pass: scores = psum + bias, maxv = max(scores)
                    maxv = spool.tile([P, 8], F32, tag="maxv")
                    nc.vector.memset(maxv[:, 1:8], -3.0e38)
                    nc.vector.tensor_tensor_reduce(
                        out=scores[:], in0=score_ps[:], in1=bias_sb[:],
                        scale=1.0, scalar=-3.0e38,
                        op0=mybir.AluOpType.add, op1=mybir.AluOpType.max,
                        accum_out=maxv[:, 0:1])
                    in_max = maxv[:]
                else:
                    nc.vector.tensor_add(scores[:], score_ps[:], bias_sb[:])
                    max8 = spool.tile([P, 8], F32, tag="max8")
                    nc.vector.max(out=max8[:], in_=scores[:])
                    in_max = max8[:]
                nc.vector.max_index(idx_acc[:, ts(t, 8)], in_max, scores[:])

            nc.sync.dma_start(o_d.ap(), idx_acc[:])

    nc.compile()
    return nc


_NC_CACHE = {}


def _get_nc(mode, n_tiles):
    key = (mode, n_tiles)
    if key not in _NC_CACHE:
        _NC_CACHE[key] = build_nc(mode, n_tiles)
    return _NC_CACHE[key]


def run(x: np.ndarray, cluster_centers: np.ndarray, mode: str = MODE,
        trace: bool = False):
    n = x.shape[0]
    n_tiles = n // (N_CORES * P)
    nc = _get_nc(mode, n_tiles)
    if mode == "fp32rr":
        x = _round_fp22(np.ascontiguousarray(x, dtype=np.float32))
        cluster_centers = _round_fp22(
            np.ascontiguousarray(cluster_centers, dtype=np.float32))
    xs = x.reshape(N_CORES, n // N_CORES, D)
    c = np.ascontiguousarray(cluster_centers, dtype=np.float32)
    in_maps = [{"x": np.ascontiguousarray(xs[i], dtype=np.float32), "cc": c}
               for i in range(N_CORES)]
    res = run_bass_kernel_spmd(nc, in_maps, core_ids=list(range(N_CORES)),
                               trace=trace)
    outs = []
    for i in range(N_CORES):
        o = res.results[i]["out"]          # [128, n_tiles*8] uint32
        idx = o[:, ::8]                    # [128 p, n_tiles t]
        outs.append(idx.T.reshape(-1))     # rows n = t*128 + p
    full = np.concatenate(outs).astype(np.int32)
    return full, res


def kernel(x: np.ndarray, cluster_centers: np.ndarray) -> np.ndarray:
    out, _ = run(np.asarray(x), np.asarray(cluster_centers))
    return out
